# revision 17
# baseline (speedup 1.0000x reference)
"""Trainium2 Bass kernel: AdaptivePrototypicalFewShotLearning.

Strategy (8-core data-parallel over N_query):
  * dist is FIXED during refinement => all 3 softmaxes depend only on the
    initial scores. One pass over queries computes all 3 weighted sums
    (soft_k.T @ qn) + counts, fused into one [128,60] lhsT matmul per tile.
  * LayerNorm g/b folded: softmax(-dist/T) is invariant to per-query ||qn||^2,
    so pass 1 needs only scores = 2*u.(g*P) + e_c  (u = raw-normalized query).
  * Tiny AllReduce (60x520 f32), refine-MLP chain replicated on-core in
    feature-major (T) layout, then pass 2 re-streams queries for logits.
"""
import math
import os
import sys

import numpy as np

sys.path.insert(0, "/opt/trn_rl_repo")

import concourse.bass as bass  # noqa: E402
import concourse.tile as tile  # noqa: E402
from concourse import bacc, mybir  # noqa: E402
from concourse.bass_utils import run_bass_kernel_spmd  # noqa: E402

F32 = mybir.dt.float32
AF = mybir.ActivationFunctionType
ALU = mybir.AluOpType

NCORES = 8
FEAT = 512
HID = 256
NCLS = 20
NSUP = 200
NQ = 200000
STEPS = 3
EPS = 1e-5

NQL = NQ // NCORES          # 25000 queries per core
ST = 512                    # queries per super-tile (1 MB DMA)
NSUB = 4                    # 128-row subtiles per super-tile
NST = math.ceil(NQL / ST)   # 49
NQP = NST * ST              # 25088 padded rows per core
PAD = NQP - NQL             # 88 pad rows -> last subtile has 40 valid rows
LAST_VALID = 128 - PAD      # 40

# dtype mode for the heavy per-tile matmuls: "f32" | "f32r" | "bf16"
MM_MODE = os.environ.get("KERNEL_MM_MODE", "f32")

LAST_EXEC_NS = None
LAST_RESULTS = None


def _dt_mm(mode):
    if mode == "bf16":
        return mybir.dt.bfloat16
    return F32


def _mm_cast(ap, mode):
    """View an f32 AP as float32r for the fast fp32 matmul path."""
    if mode == "f32r":
        return ap.bitcast(mybir.dt.float32r)
    return ap


def build_graph(gamma2: float, bnorm2: float, dist_temp: float, mm_mode: str,
                debug: bool = False):
    nc = bacc.Bacc(
        "TRN2",
        target_bir_lowering=False,
        debug=False,
        num_devices=NCORES,
    )
    mmdt = _dt_mm(mm_mode)          # storage dtype of transpose/score operands
    bf = mm_mode == "bf16"

    # ---- DRAM parameters --------------------------------------------------
    def inp(name, shape):
        return nc.dram_tensor(name, shape, F32, kind="ExternalInput").ap()

    qx = inp("qx", [NQP, FEAT])
    sup = inp("sup", [256, FEAT])
    oh = inp("oh", [256, NCLS])
    iden = inp("iden", [128, 128])
    ones8_d = inp("ones8", [128, 8])
    mask_d = inp("mask", [128, 1])
    g_d = inp("g", [FEAT])
    b_d = inp("b", [FEAT])
    p2g2_d = inp("p2g2", [FEAT])    # 2*g^2
    p2gb_d = inp("p2gb", [FEAT])    # 2*g*b
    m2g2_d = inp("m2g2", [FEAT])    # -2*g^2
    m2gb_d = inp("m2gb", [FEAT])    # -2*g*b
    brows_d = inp("brows", [1, 4, 128])
    w1_d = inp("w1", [FEAT, HID])
    b1_d = inp("b1", [HID])
    w2_d = inp("w2", [HID, FEAT])
    b2_d = inp("b2", [FEAT])
    rw1_d = inp("rw1", [2 * FEAT, FEAT])
    rb1_d = inp("rb1", [FEAT])
    rw2_d = inp("rw2", [FEAT, FEAT])
    rb2s_d = inp("rb2s", [FEAT])    # 0.1 * rf_b2
    out_d = nc.dram_tensor("out", [NQP, NCLS], F32, kind="ExternalOutput").ap()
    dbg = {}
    if debug:
        for nm, shp in [("dbg_P", [128, 4, NCLS]), ("dbg_A", [128, 4, NCLS]),
                        ("dbg_e", [1, NCLS]), ("dbg_M", [60, 520]),
                        ("dbg_wmT", [128, 4, 60]), ("dbg_R", [128, 4, NCLS]),
                        ("dbg_A2e", [128, 4, 21]), ("dbg_e2", [1, 21]),
                        ("dbg_sc", [128, 24]), ("dbg_u", [128, FEAT]),
                        ("dbg_utT", [128, FEAT]), ("dbg_soft", [128, 64])]:
            dbg[nm] = nc.dram_tensor(nm, shp, F32, kind="ExternalOutput").ap()

    qx_r = qx.rearrange("(t c p) f -> t p c f", c=NSUB, p=128)
    out_r = out_d.rearrange("(t c p) n -> t p c n", c=NSUB, p=128)

    with tile.TileContext(nc) as tc:
        with tc.tile_pool(name="persist", bufs=1) as pp, \
             tc.tile_pool(name="dram", bufs=1, space="DRAM") as dp:
            # ---- load constants/weights into SBUF -------------------------
            w1 = pp.tile([128, 4, HID], F32)
            nc.sync.dma_start(w1, w1_d.rearrange("(k p) n -> p k n", p=128))
            w2 = pp.tile([128, 2, FEAT], F32)
            nc.sync.dma_start(w2, w2_d.rearrange("(k p) n -> p k n", p=128))
            rw1 = pp.tile([128, 8, FEAT], F32)
            nc.sync.dma_start(rw1, rw1_d.rearrange("(k p) n -> p k n", p=128))
            rw2 = pp.tile([128, 4, FEAT], F32)
            nc.sync.dma_start(rw2, rw2_d.rearrange("(k p) n -> p k n", p=128))

            def colvec(src, k):
                t = pp.tile([128, k], F32, tag=f"cv_{src.tensor.name}")
                nc.sync.dma_start(t, src.rearrange("(k p) -> p k", p=128))
                return t

            b1T = colvec(b1_d, 2)
            b2T = colvec(b2_d, 4)
            rb1T = colvec(rb1_d, 4)
            rb2sT = colvec(rb2s_d, 4)
            g_sb = colvec(g_d, 4)
            b_sb = colvec(b_d, 4)
            p2g2s = colvec(p2g2_d, 4)
            p2gbs = colvec(p2gb_d, 4)
            m2g2s = colvec(m2g2_d, 4)
            m2gbs = colvec(m2gb_d, 4)

            ident = pp.tile([128, 128], F32)
            nc.sync.dma_start(ident, iden)
            ones8 = pp.tile([128, 8], mmdt)
            nc.sync.dma_start(ones8, ones8_d) if not bf else None
            if bf:
                o8f = pp.tile([128, 8], F32)
                nc.sync.dma_start(o8f, ones8_d)
                nc.scalar.copy(ones8, o8f)
            mask = pp.tile([128, 1], F32)
            nc.sync.dma_start(mask, mask_d)
            brows = pp.tile([1, 4, 128], F32)
            nc.sync.dma_start(brows, brows_d)
            oh_sb = pp.tile([128, 2, NCLS], F32)
            nc.sync.dma_start(oh_sb, oh.rearrange("(k p) c -> p k c", p=128))
            sup_sb = pp.tile([128, 2, FEAT], F32)
            nc.sync.dma_start(sup_sb, sup.rearrange("(k p) f -> p k f", p=128))

            onescol = pp.tile([128, 1], F32)
            nc.vector.memset(onescol, 1.0)
            ones1f_su = pp.tile([1, 128], F32)
            nc.vector.memset(ones1f_su, 1.0)
            epsc = pp.tile([128, 1], F32)
            nc.vector.memset(epsc, EPS)
            zeroc = pp.tile([128, 1], F32)
            nc.vector.memset(zeroc, 0.0)
            ones1r = pp.tile([1, 128], mmdt)
            nc.vector.memset(ones1r, 1.0)

            # persistent results of setup
            A_T = pp.tile([128, 4, NCLS], mmdt)      # 2*g*P, feature-major
            e_sb = pp.tile([1, NCLS], mmdt)          # -||P_c||^2 + 2 b.P_c
            P_T = pp.tile([128, 4, NCLS], F32)       # protos (g,b applied)
            wmT = pp.tile([128, 4, 60], F32)         # weighted means, T layout
            A2e = pp.tile([128, 4, 21], mmdt)        # [-2*g*R | 2gb]
            e2_sb = pp.tile([1, 21], mmdt)
            e_bc = pp.tile([128, NCLS], F32)         # e broadcast over partitions
            e2_bc = pp.tile([128, 21], F32)

            # ======================= SETUP: protos =========================
            with tc.tile_pool(name="su_ps", bufs=1, space="PSUM") as sps, \
                 tc.tile_pool(name="su_sb", bufs=2) as ssb:
                # LN support (2 row-chunks of 128)
                st6 = ssb.tile([128, 2, 6], F32)
                mv = ssb.tile([128, 2, 2], F32)
                for k in range(2):
                    nc.vector.bn_stats(st6[:, k, :], sup_sb[:, k, :])
                    nc.vector.bn_aggr(mv[:, k, :], st6[:, k, :])
                sd = ssb.tile([128, 2], F32)
                nc.scalar.activation(sd, mv[:, :, 1], AF.Sqrt, bias=epsc, scale=1.0)
                rr = ssb.tile([128, 2], F32)
                nc.vector.reciprocal(rr, sd)
                us = ssb.tile([128, 2, FEAT], F32)
                for k in range(2):
                    nmr = ssb.tile([128, 1], F32)
                    nc.vector.tensor_tensor(
                        out=nmr, in0=mv[:, k, 0:1], in1=rr[:, k:k + 1], op=ALU.mult)
                    nc.vector.tensor_scalar_mul(nmr, nmr, -1.0)
                    nc.gpsimd.tensor_scalar(
                        out=us[:, k, :], in0=sup_sb[:, k, :],
                        scalar1=rr[:, k:k + 1], scalar2=nmr,
                        op0=ALU.mult, op1=ALU.add)
                # cmeanT[f, c] = sum_s us[s, f] * oh[s, c]
                cmP = sps.tile([128, 4, NCLS], F32)
                for m in range(4):
                    for k in range(2):
                        nc.tensor.matmul(
                            cmP[:, m, :], lhsT=us[:, k, 128 * m:128 * (m + 1)],
                            rhs=oh_sb[:, k, :], start=(k == 0), stop=(k == 1))
                cmT = ssb.tile([128, 4, NCLS], F32)
                nc.scalar.copy(cmT, cmP)
                # h1T = relu(W1^T cmean + b1)
                h1P = sps.tile([128, 2, NCLS], F32)
                for m in range(2):
                    for k in range(4):
                        nc.tensor.matmul(
                            h1P[:, m, :], lhsT=w1[:, k, 128 * m:128 * (m + 1)],
                            rhs=cmT[:, k, :], start=(k == 0), stop=(k == 3))
                h1T = ssb.tile([128, 2, NCLS], F32)
                for m in range(2):
                    nc.scalar.activation(
                        h1T[:, m, :], h1P[:, m, :], AF.Relu,
                        bias=b1T[:, m:m + 1], scale=1.0)
                # p0T = W2^T h1 + b2
                p0P = sps.tile([128, 4, NCLS], F32)
                for m in range(4):
                    for k in range(2):
                        nc.tensor.matmul(
                            p0P[:, m, :], lhsT=w2[:, k, 128 * m:128 * (m + 1)],
                            rhs=h1T[:, k, :], start=(k == 0), stop=(k == 1))
                p0T = ssb.tile([128, 4, NCLS], F32)
                for m in range(4):
                    nc.scalar.activation(
                        p0T[:, m, :], p0P[:, m, :], AF.Identity,
                        bias=b2T[:, m:m + 1], scale=1.0)

                upT = ssb.tile([128, 4, NCLS], F32)
                _ln_over_partitions(nc, tc, sps, ssb, p0T, upT, onescol, ones1f_su,
                                    epsc, zeroc)
                for m in range(4):
                    nc.scalar.activation(
                        P_T[:, m, :], upT[:, m, :], AF.Identity,
                        bias=b_sb[:, m:m + 1], scale=g_sb[:, m:m + 1])
                    nc.scalar.activation(
                        A_T[:, m, :], upT[:, m, :], AF.Identity,
                        bias=p2gbs[:, m:m + 1], scale=p2g2s[:, m:m + 1])
                _class_consts(nc, sps, ssb, P_T, b_sb, onescol, e_sb,
                              scale_bp=2.0, sign_pp=-1.0, ident=ident,
                              zeroc=zeroc)
                ebP = sps.tile([128, 64], F32, tag="ebP")
                nc.tensor.matmul(ebP[:, 0:NCLS], lhsT=ones1f_su,
                                 rhs=e_sb if not bf else None, start=True,
                                 stop=True) if not bf else None
                if bf:
                    e_f32 = ssb.tile([1, NCLS], F32)
                    nc.scalar.copy(e_f32, e_sb)
                    nc.tensor.matmul(ebP[:, 0:NCLS], lhsT=ones1f_su, rhs=e_f32,
                                     start=True, stop=True)
                nc.scalar.copy(e_bc, ebP[:, 0:NCLS])
                if debug:
                    nc.sync.dma_start(dbg["dbg_P"], P_T)
                    nc.sync.dma_start(dbg["dbg_A"], A_T)
                    nc.sync.dma_start(dbg["dbg_e"], e_sb)

            # ======================= PASS 1 ================================
            with tc.tile_pool(name="p1x", bufs=3) as xp, \
                 tc.tile_pool(name="p1u", bufs=2) as up_, \
                 tc.tile_pool(name="p1ut", bufs=3) as utp, \
                 tc.tile_pool(name="p1sm", bufs=16) as smp, \
                 tc.tile_pool(name="p1soft", bufs=2) as sfp, \
                 tc.tile_pool(name="p1tp", bufs=2, space="PSUM") as tpp, \
                 tc.tile_pool(name="p1sc", bufs=2, space="PSUM") as scp, \
                 tc.tile_pool(name="p1acc", bufs=1, space="PSUM") as accp:
                pmacc = accp.tile([60, FEAT], F32)
                pws = accp.tile([60, 8], F32)
                nsub_tot = NST * NSUB
                for st_i in range(NST):
                    xt = xp.tile([128, NSUB, FEAT], F32)
                    nc.sync.dma_start(xt, qx_r[st_i])
                    r4, nmr4 = _ln_stats(nc, smp, xt, epsc)
                    ut = up_.tile([128, NSUB, FEAT], mmdt)
                    for c in range(NSUB):
                        nc.gpsimd.tensor_scalar(
                            out=ut[:, c, :], in0=xt[:, c, :],
                            scalar1=r4[:, c:c + 1], scalar2=nmr4[:, c:c + 1],
                            op0=ALU.mult, op1=ALU.add)
                    for c in range(NSUB):
                        isub = st_i * NSUB + c
                        utT = _transpose512(nc, tpp, utp, ut[:, c, :], ident,
                                            mmdt, mm_mode)
                        psc = scp.tile([128, 24], F32)
                        for j in range(4):
                            nc.tensor.matmul(
                                psc[:, 0:NCLS],
                                lhsT=_mm_cast(utT[j], mm_mode),
                                rhs=_mm_cast(A_T[:, j, :], mm_mode),
                                start=(j == 0), stop=(j == 3))
                        if debug and isub == 0:
                            scs = pp.tile([128, 24], F32)
                            nc.scalar.copy(scs, psc)
                            nc.sync.dma_start(dbg["dbg_sc"], scs)
                        sc2 = smp.tile([128, NCLS], F32, tag="sc2")
                        nc.vector.tensor_tensor(
                            out=sc2, in0=psc[:, 0:NCLS], in1=e_bc, op=ALU.add)
                        mx = smp.tile([128, 1], F32)
                        nc.vector.tensor_reduce(
                            mx, sc2, axis=mybir.AxisListType.X, op=ALU.max)
                        sc0 = smp.tile([128, NCLS], F32, tag="sc0")
                        nc.vector.tensor_scalar_sub(sc0, sc2, mx)
                        soft = sfp.tile([128, 64], mmdt)
                        for k in range(STEPS):
                            ek = smp.tile([128, NCLS], F32)
                            sk = smp.tile([128, 1], F32)
                            nc.scalar.activation(
                                ek, sc0, AF.Exp,
                                bias=zeroc, scale=1.0 / float(k + 1),
                                accum_out=sk)
                            rk = smp.tile([128, 1], F32)
                            nc.vector.reciprocal(rk, sk)
                            nc.gpsimd.tensor_scalar_mul(
                                soft[:, NCLS * k:NCLS * (k + 1)], ek, rk)
                        if debug and isub == 0:
                            nc.sync.dma_start(dbg["dbg_soft"], soft)
                        if st_i == NST - 1 and c == NSUB - 1:
                            nc.gpsimd.tensor_scalar_mul(
                                soft[:, 0:60], soft[:, 0:60], mask)
                        nc.tensor.matmul(
                            pmacc, lhsT=_mm_cast(soft[:, 0:60], mm_mode),
                            rhs=_mm_cast(ut[:, c, :], mm_mode),
                            start=(isub == 0), stop=(isub == nsub_tot - 1))
                        nc.tensor.matmul(
                            pws[:, 0:8], lhsT=_mm_cast(soft[:, 0:60], mm_mode),
                            rhs=ones8, start=(isub == 0),
                            stop=(isub == nsub_tot - 1))
                # ship partial sums to the collective (PSUM -> SBUF -> DRAM)
                bin_ = dp.tile([60, 520], F32)
                bout = dp.tile([60, 520], F32)
                stage = pp.tile([60, 520], F32)
                nc.scalar.copy(stage[:, 0:FEAT], pmacc)
                nc.vector.tensor_copy(out=stage[:, FEAT:520], in_=pws)
                nc.sync.dma_start(bin_, stage)

            nc.gpsimd.collective_compute(
                "AllReduce", ALU.add,
                replica_groups=[list(range(NCORES))],
                ins=[bin_.opt()], outs=[bout.opt()])

            # ================== MID: wmeans + refine chain =================
            with tc.tile_pool(name="md_ps", bufs=1, space="PSUM") as mps, \
                 tc.tile_pool(name="md_sb", bufs=2) as msb:
                Mw = msb.tile([60, 520], F32)
                nc.sync.dma_start(Mw, bout)
                ws = msb.tile([60, 1], F32)
                nc.vector.tensor_scalar_max(ws, Mw[:, FEAT:FEAT + 1], 1e-6)
                rw60 = msb.tile([60, 1], F32)
                nc.vector.reciprocal(rw60, ws)
                sr = msb.tile([60, 1], F32)
                nc.vector.tensor_tensor(
                    out=sr, in0=Mw[:, FEAT:FEAT + 1], in1=rw60, op=ALU.mult)
                # transpose M and the two per-class vectors
                mtP = mps.tile([128, 4, 60], F32)
                for j in range(4):
                    nc.tensor.transpose(
                        mtP[:, j, :], Mw[0:60, 128 * j:128 * (j + 1)],
                        ident[0:60, 0:60])
                rsP = mps.tile([1, 128], F32)
                nc.tensor.transpose(rsP[0:1, 0:60], rw60, ident[0:60, 0:60])
                nc.tensor.transpose(rsP[0:1, 64:124], sr, ident[0:60, 0:60])
                rsT = msb.tile([1, 128], F32)
                nc.scalar.copy(rsT[0:1, 0:60], rsP[0:1, 0:60])
                nc.scalar.copy(rsT[0:1, 64:124], rsP[0:1, 64:124])
                # broadcast across partitions via K=1 matmuls
                ones1f = msb.tile([1, 128], F32)
                nc.vector.memset(ones1f, 1.0)
                bcP = mps.tile([128, 128], F32)
                nc.tensor.matmul(bcP[:, 0:60], lhsT=ones1f, rhs=rsT[0:1, 0:60],
                                 start=True, stop=True)
                bsrP = mps.tile([128, 4, 60], F32)
                for m in range(4):
                    nc.tensor.matmul(
                        bsrP[:, m, :], lhsT=brows[0:1, m, :],
                        rhs=rsT[0:1, 64:124], start=True, stop=True)
                rwbc = msb.tile([128, 60], F32)
                nc.scalar.copy(rwbc, bcP[:, 0:60])
                bsr = msb.tile([128, 4, 60], F32)
                nc.scalar.copy(bsr, bsrP)
                for m in range(4):
                    t1 = msb.tile([128, 60], F32)
                    nc.vector.tensor_tensor(
                        out=t1, in0=mtP[:, m, :], in1=rwbc, op=ALU.mult)
                    t2 = msb.tile([128, 60], F32)
                    nc.scalar.activation(t2, t1, AF.Identity,
                                         bias=zeroc, scale=g_sb[:, m:m + 1])
                    nc.vector.tensor_tensor(
                        out=wmT[:, m, :], in0=t2, in1=bsr[:, m, :], op=ALU.add)

                if debug:
                    nc.sync.dma_start(dbg["dbg_M"], Mw)
                refT = msb.tile([128, 4, NCLS], F32)
                nc.scalar.copy(refT, P_T)
                for step in range(STEPS):
                    hP = mps.tile([128, 4, NCLS], F32)
                    for m in range(4):
                        for kk in range(8):
                            rhs = (refT[:, kk, :] if kk < 4 else
                                   wmT[:, kk - 4, NCLS * step:NCLS * (step + 1)])
                            nc.tensor.matmul(
                                hP[:, m, :],
                                lhsT=rw1[:, kk, 128 * m:128 * (m + 1)],
                                rhs=rhs, start=(kk == 0), stop=(kk == 7))
                    hT = msb.tile([128, 4, NCLS], F32)
                    for m in range(4):
                        nc.scalar.activation(hT[:, m, :], hP[:, m, :], AF.Relu,
                                             bias=rb1T[:, m:m + 1], scale=1.0)
                    dP = mps.tile([128, 4, NCLS], F32)
                    for m in range(4):
                        for kk in range(4):
                            nc.tensor.matmul(
                                dP[:, m, :],
                                lhsT=rw2[:, kk, 128 * m:128 * (m + 1)],
                                rhs=hT[:, kk, :], start=(kk == 0), stop=(kk == 3))
                    refT_new = msb.tile([128, 4, NCLS], F32)
                    for m in range(4):
                        t = msb.tile([128, NCLS], F32)
                        nc.scalar.activation(t, dP[:, m, :], AF.Identity,
                                             bias=rb2sT[:, m:m + 1], scale=0.1)
                        nc.vector.tensor_tensor(
                            out=refT_new[:, m, :], in0=refT[:, m, :], in1=t,
                            op=ALU.add)
                    refT = refT_new

                upRT = msb.tile([128, 4, NCLS], F32)
                _ln_over_partitions(nc, tc, mps, msb, refT, upRT, onescol, ones1f,
                                    epsc, zeroc)
                R_T = msb.tile([128, 4, NCLS], F32)
                for m in range(4):
                    nc.scalar.activation(
                        R_T[:, m, :], upRT[:, m, :], AF.Identity,
                        bias=b_sb[:, m:m + 1], scale=g_sb[:, m:m + 1])
                    nc.scalar.activation(
                        A2e[:, m, 0:NCLS], upRT[:, m, :], AF.Identity,
                        bias=m2gbs[:, m:m + 1], scale=m2g2s[:, m:m + 1])
                    nc.vector.tensor_copy(
                        out=A2e[:, m, NCLS:21], in_=p2gbs[:, m:m + 1])
                _class_consts(nc, mps, msb, R_T, b_sb, onescol, e2_sb[0:1, 0:NCLS],
                              scale_bp=-2.0, sign_pp=1.0, ident=ident,
                              zeroc=zeroc)
                nc.vector.memset(e2_sb[0:1, NCLS:21], bnorm2)
                e2bP = mps.tile([128, 64], F32, tag="bcP")
                if bf:
                    e2_f32 = msb.tile([1, 21], F32)
                    nc.scalar.copy(e2_f32, e2_sb)
                    nc.tensor.matmul(e2bP[:, 0:21], lhsT=ones1f,
                                     rhs=e2_f32, start=True, stop=True)
                else:
                    nc.tensor.matmul(e2bP[:, 0:21], lhsT=ones1f,
                                     rhs=e2_sb, start=True, stop=True)
                nc.scalar.copy(e2_bc, e2bP[:, 0:21])
                if debug:
                    nc.sync.dma_start(dbg["dbg_wmT"], wmT)
                    nc.sync.dma_start(dbg["dbg_R"], R_T)
                    nc.sync.dma_start(dbg["dbg_A2e"], A2e)
                    nc.sync.dma_start(dbg["dbg_e2"], e2_sb)

            # ======================= PASS 2 ================================
            with tc.tile_pool(name="p2x", bufs=3) as xp2, \
                 tc.tile_pool(name="p2u", bufs=2) as up2, \
                 tc.tile_pool(name="p2ut", bufs=3) as utp2, \
                 tc.tile_pool(name="p2sm", bufs=16) as smp2, \
                 tc.tile_pool(name="p2l", bufs=2) as lp2, \
                 tc.tile_pool(name="p2tp", bufs=2, space="PSUM") as tpp2, \
                 tc.tile_pool(name="p2z", bufs=2, space="PSUM") as zp2:
                for st_i in range(NST):
                    xt = xp2.tile([128, NSUB, FEAT], F32)
                    nc.sync.dma_start(xt, qx_r[st_i])
                    r4, nmr4, v4 = _ln_stats(nc, smp2, xt, epsc, want_v=True)
                    nrm4 = smp2.tile([128, NSUB], F32)
                    nc.vector.tensor_tensor(out=nrm4, in0=v4, in1=r4, op=ALU.mult)
                    nc.vector.tensor_tensor(out=nrm4, in0=nrm4, in1=r4, op=ALU.mult)
                    nc.vector.tensor_scalar_mul(nrm4, nrm4, float(FEAT) * gamma2)
                    ut = up2.tile([128, NSUB, FEAT], mmdt)
                    for c in range(NSUB):
                        nc.gpsimd.tensor_scalar(
                            out=ut[:, c, :], in0=xt[:, c, :],
                            scalar1=r4[:, c:c + 1], scalar2=nmr4[:, c:c + 1],
                            op0=ALU.mult, op1=ALU.add)
                    lt = lp2.tile([128, NSUB, NCLS], F32)
                    for c in range(NSUB):
                        utT = _transpose512(nc, tpp2, utp2, ut[:, c, :], ident,
                                            mmdt, mm_mode)
                        pz = zp2.tile([128, 24], F32)
                        for j in range(4):
                            nc.tensor.matmul(
                                pz[:, 0:21],
                                lhsT=_mm_cast(utT[j], mm_mode),
                                rhs=_mm_cast(A2e[:, j, :], mm_mode),
                                start=(j == 0), stop=(j == 3))
                        z2 = smp2.tile([128, 21], F32, tag="z2")
                        nc.vector.tensor_tensor(
                            out=z2, in0=pz[:, 0:21], in1=e2_bc, op=ALU.add)
                        bias2 = smp2.tile([128, 1], F32)
                        nc.vector.tensor_tensor(
                            out=bias2, in0=nrm4[:, c:c + 1], in1=z2[:, NCLS:21],
                            op=ALU.add)
                        nc.scalar.activation(lt[:, c, :], z2[:, 0:NCLS], AF.Relu,
                                             bias=bias2, scale=1.0)
                    nc.gpsimd.tensor_scalar_mul(lt, lt, -float(dist_temp))
                    nc.sync.dma_start(out_r[st_i], lt)

    nc.compile()
    return nc


def _ln_stats(nc, pool, xt, epsc, want_v=False):
    """bn_stats LN statistics for a [128, NSUB, FEAT] tile.

    Returns r (rstd), nmr (-mean*rstd) as [128, NSUB], optionally v."""
    st6 = pool.tile([128, NSUB, 6], F32)
    mv = pool.tile([128, NSUB, 2], F32)
    for c in range(NSUB):
        nc.vector.bn_stats(st6[:, c, :], xt[:, c, :])
        nc.vector.bn_aggr(mv[:, c, :], st6[:, c, :])
    sd = pool.tile([128, NSUB], F32)
    nc.scalar.activation(sd, mv[:, :, 1], AF.Sqrt, bias=epsc, scale=1.0)
    r4 = pool.tile([128, NSUB], F32)
    nc.vector.reciprocal(r4, sd)
    nmr4 = pool.tile([128, NSUB], F32)
    nc.vector.tensor_tensor(out=nmr4, in0=mv[:, :, 0], in1=r4, op=ALU.mult)
    nc.vector.tensor_scalar_mul(nmr4, nmr4, -1.0)
    if want_v:
        v4 = pool.tile([128, NSUB], F32)
        nc.vector.tensor_copy(out=v4, in_=mv[:, :, 1])
        return r4, nmr4, v4
    return r4, nmr4


def _transpose512(nc, psum_pool, sbuf_pool, src, ident, mmdt, mm_mode):
    """[128q, 512f] SBUF -> 4 chunk APs [128f_j, 128q] (feature-major)."""
    if mm_mode == "bf16":
        utT = sbuf_pool.tile([128, 4, 128], mmdt, tag="utT")
        nc.scalar.dma_start_transpose(utT, src)
        return [utT[:, j, :] for j in range(4)]
    pT = psum_pool.tile([128, FEAT], mmdt)
    for j in range(4):
        inp = src[:, 128 * j:128 * (j + 1)]
        nc.tensor.transpose(pT[:, 128 * j:128 * (j + 1)], inp, ident)
    utT = sbuf_pool.tile([128, FEAT], mmdt, tag="utT")
    nc.scalar.copy(utT[:, 0:256], pT[:, 0:256])
    nc.vector.tensor_copy(out=utT[:, 256:FEAT], in_=pT[:, 256:FEAT])
    return [utT[:, 128 * j:128 * (j + 1)] for j in range(4)]


def _ln_over_partitions(nc, tc, psum_pool, sbuf_pool, xT, outT, onescol, ones1f,
                        epsc, zeroc):
    """LayerNorm over the partition axis for [128, 4, NCLS] (512 features of
    NCLS classes, feature-major). outT = (x - mean) * rstd."""
    sP = psum_pool.tile([1, 64], F32)
    x2 = sbuf_pool.tile([128, 4, NCLS], F32)
    nc.scalar.activation(x2, xT, AF.Square, bias=zeroc, scale=1.0)
    for m in range(4):
        nc.tensor.matmul(sP[0:1, 0:NCLS], lhsT=onescol, rhs=xT[:, m, :],
                         start=(m == 0), stop=(m == 3))
    for m in range(4):
        nc.tensor.matmul(sP[0:1, 32:32 + NCLS], lhsT=onescol, rhs=x2[:, m, :],
                         start=(m == 0), stop=(m == 3))
    mrow = sbuf_pool.tile([1, NCLS], F32)
    nc.vector.tensor_scalar_mul(mrow, sP[0:1, 0:NCLS], 1.0 / FEAT)
    m2 = sbuf_pool.tile([1, NCLS], F32)
    nc.scalar.activation(m2, mrow, AF.Square, bias=zeroc[0:1], scale=1.0)
    vrow = sbuf_pool.tile([1, NCLS], F32)
    nc.vector.tensor_scalar(out=vrow, in0=sP[0:1, 32:32 + NCLS],
                            scalar1=1.0 / FEAT, scalar2=None, op0=ALU.mult)
    nc.vector.tensor_tensor(out=vrow, in0=vrow, in1=m2, op=ALU.subtract)
    sdr = sbuf_pool.tile([1, NCLS], F32)
    nc.scalar.activation(sdr, vrow, AF.Sqrt, bias=epsc[0:1], scale=1.0)
    rrow = sbuf_pool.tile([1, NCLS], F32)
    nc.vector.reciprocal(rrow, sdr)
    bcP = psum_pool.tile([128, 64], F32)
    nc.tensor.matmul(bcP[:, 0:NCLS], lhsT=ones1f, rhs=mrow, start=True, stop=True)
    nc.tensor.matmul(bcP[:, 32:32 + NCLS], lhsT=ones1f, rhs=rrow,
                     start=True, stop=True)
    mbc = sbuf_pool.tile([128, NCLS], F32)
    nc.scalar.copy(mbc, bcP[:, 0:NCLS])
    rbc = sbuf_pool.tile([128, NCLS], F32)
    nc.scalar.copy(rbc, bcP[:, 32:32 + NCLS])
    for m in range(4):
        nc.vector.tensor_tensor(out=outT[:, m, :], in0=xT[:, m, :], in1=mbc,
                                op=ALU.subtract)
        nc.vector.tensor_tensor(out=outT[:, m, :], in0=outT[:, m, :], in1=rbc,
                                op=ALU.mult)


def _class_consts(nc, psum_pool, sbuf_pool, P_T, b_sb, onescol, e_out,
                  scale_bp, sign_pp, ident, zeroc):
    """e_out[1, NCLS] = sign_pp * ||P_c||^2 + scale_bp * (b . P_c)."""
    PT2 = sbuf_pool.tile([128, 4, NCLS], F32)
    nc.scalar.activation(PT2, P_T, AF.Square, bias=zeroc, scale=1.0)
    eP = psum_pool.tile([1, 64], F32)
    for m in range(4):
        nc.tensor.matmul(eP[0:1, 0:NCLS], lhsT=onescol, rhs=PT2[:, m, :],
                         start=(m == 0), stop=(m == 3))
    for m in range(4):
        nc.tensor.matmul(eP[0:1, 32:32 + NCLS], lhsT=b_sb[:, m:m + 1],
                         rhs=P_T[:, m, :], start=(m == 0), stop=(m == 3))
    t = sbuf_pool.tile([1, NCLS], F32)
    nc.vector.tensor_scalar_mul(t, eP[0:1, 32:32 + NCLS], scale_bp)
    t2 = sbuf_pool.tile([1, NCLS], F32)
    nc.vector.tensor_scalar_mul(t2, eP[0:1, 0:NCLS], sign_pp)
    nc.vector.tensor_tensor(out=e_out, in0=t2, in1=t, op=ALU.add)


def kernel(_debug=False, **inputs) -> np.ndarray:
    global LAST_EXEC_NS, LAST_RESULTS
    f32 = np.float32
    qf = np.asarray(inputs["query_features"], f32)
    sf = np.asarray(inputs["support_features"], f32)
    lab = np.asarray(inputs["support_labels"]).astype(np.int64)
    g = np.asarray(inputs["ln_g"], f32)
    b = np.asarray(inputs["ln_b"], f32)
    temp = float(np.asarray(inputs["dist_temp"]))

    assert np.allclose(g, g[0]), "kernel fast path assumes constant ln_g"
    gamma2 = float(g[0]) ** 2
    bnorm2 = float(b @ b)

    nc = build_graph(gamma2, bnorm2, temp, MM_MODE, debug=_debug)

    sup = np.zeros((256, FEAT), f32)
    sup[:NSUP] = sf
    cnt = np.bincount(lab, minlength=NCLS).astype(f32)
    oh = np.zeros((256, NCLS), f32)
    oh[np.arange(NSUP), lab] = 1.0 / np.maximum(cnt, 1.0)[lab]
    mask = (np.arange(128) < LAST_VALID).astype(f32).reshape(128, 1)

    common = {
        "sup": sup, "oh": oh,
        "iden": np.eye(128, dtype=f32),
        "ones8": np.ones((128, 8), f32),
        "mask": mask,
        "g": g, "b": b,
        "p2g2": (2.0 * g * g).astype(f32), "p2gb": (2.0 * g * b).astype(f32),
        "m2g2": (-2.0 * g * g).astype(f32), "m2gb": (-2.0 * g * b).astype(f32),
        "brows": b.reshape(1, 4, 128).copy(),
        "w1": np.asarray(inputs["pg_w1"], f32), "b1": np.asarray(inputs["pg_b1"], f32),
        "w2": np.asarray(inputs["pg_w2"], f32), "b2": np.asarray(inputs["pg_b2"], f32),
        "rw1": np.asarray(inputs["rf_w1"], f32), "rb1": np.asarray(inputs["rf_b1"], f32),
        "rw2": np.asarray(inputs["rf_w2"], f32),
        "rb2s": (0.1 * np.asarray(inputs["rf_b2"], f32)).astype(f32),
    }
    in_maps = []
    for i in range(NCORES):
        shard = np.zeros((NQP, FEAT), f32)
        shard[:NQL] = qf[i * NQL:(i + 1) * NQL]
        in_maps.append({"qx": shard, **common})

    trace = bool(int(os.environ.get("KERNEL_TRACE", "0")))
    res = run_bass_kernel_spmd(nc, in_maps, list(range(NCORES)), trace=trace)
    LAST_EXEC_NS = res.exec_time_ns
    LAST_RESULTS = res
    out = np.concatenate([res.results[i]["out"][:NQL] for i in range(NCORES)], 0)
    return out


# revision 18
# speedup vs baseline: 1.3752x; 1.3752x over previous
"""Trainium2 Bass kernel: AdaptivePrototypicalFewShotLearning.

Strategy (8-core data-parallel over N_query):
  * dist is FIXED during refinement => all 3 softmaxes depend only on the
    initial scores. One pass over queries computes all 3 weighted sums
    (soft_k.T @ qn) + counts, fused into one [128,60] lhsT matmul per tile.
  * LayerNorm g/b folded: softmax(-dist/T) is invariant to per-query ||qn||^2,
    so pass 1 needs only scores = 2*u.(g*P) + e_c  (u = raw-normalized query).
  * Tiny AllReduce (60x520 f32), refine-MLP chain replicated on-core in
    feature-major (T) layout, then pass 2 re-streams queries for logits.
"""
import math
import os
import sys

import numpy as np

sys.path.insert(0, "/opt/trn_rl_repo")

import concourse.bass as bass  # noqa: E402
import concourse.tile as tile  # noqa: E402
from concourse import bacc, mybir  # noqa: E402
from concourse.bass_utils import run_bass_kernel_spmd  # noqa: E402

F32 = mybir.dt.float32
AF = mybir.ActivationFunctionType
ALU = mybir.AluOpType

NCORES = 8
FEAT = 512
HID = 256
NCLS = 20
NSUP = 200
NQ = 200000
STEPS = 3
EPS = 1e-5

NQL = NQ // NCORES          # 25000 queries per core
ST = 512                    # queries per super-tile (1 MB DMA)
NSUB = 4                    # 128-row subtiles per super-tile
NST = math.ceil(NQL / ST)   # 49
NQP = NST * ST              # 25088 padded rows per core
PAD = NQP - NQL             # 88 pad rows -> last subtile has 40 valid rows
LAST_VALID = 128 - PAD      # 40

# dtype mode for the heavy per-tile matmuls: "f32" | "f32r" | "bf16"
MM_MODE = os.environ.get("KERNEL_MM_MODE", "f32")

LAST_EXEC_NS = None
LAST_RESULTS = None


def _dt_mm(mode):
    if mode == "bf16":
        return mybir.dt.bfloat16
    return F32


def _mm_cast(ap, mode):
    """View an f32 AP as float32r for the fast fp32 matmul path."""
    if mode == "f32r":
        return ap.bitcast(mybir.dt.float32r)
    return ap


def build_graph(gamma2: float, bnorm2: float, dist_temp: float, mm_mode: str,
                debug: bool = False):
    nc = bacc.Bacc(
        "TRN2",
        target_bir_lowering=False,
        debug=False,
        num_devices=NCORES,
    )
    mmdt = _dt_mm(mm_mode)          # storage dtype of transpose/score operands
    bf = mm_mode == "bf16"

    # ---- DRAM parameters --------------------------------------------------
    def inp(name, shape):
        return nc.dram_tensor(name, shape, F32, kind="ExternalInput").ap()

    qx = inp("qx", [NQP, FEAT])
    sup = inp("sup", [256, FEAT])
    oh = inp("oh", [256, NCLS])
    iden = inp("iden", [128, 128])
    ones8_d = inp("ones8", [128, 8])
    mask_d = inp("mask", [128, 1])
    g_d = inp("g", [FEAT])
    b_d = inp("b", [FEAT])
    p2g2_d = inp("p2g2", [FEAT])    # 2*g^2
    p2gb_d = inp("p2gb", [FEAT])    # 2*g*b
    m2g2_d = inp("m2g2", [FEAT])    # -2*g^2
    m2gb_d = inp("m2gb", [FEAT])    # -2*g*b
    brows_d = inp("brows", [1, 4, 128])
    w1_d = inp("w1", [FEAT, HID])
    b1_d = inp("b1", [HID])
    w2_d = inp("w2", [HID, FEAT])
    b2_d = inp("b2", [FEAT])
    rw1_d = inp("rw1", [2 * FEAT, FEAT])
    rb1_d = inp("rb1", [FEAT])
    rw2_d = inp("rw2", [FEAT, FEAT])
    rb2s_d = inp("rb2s", [FEAT])    # 0.1 * rf_b2
    out_d = nc.dram_tensor("out", [NQP, NCLS], F32, kind="ExternalOutput").ap()
    dbg = {}
    if debug:
        for nm, shp in [("dbg_P", [128, 4, NCLS]), ("dbg_A", [128, 4, NCLS]),
                        ("dbg_e", [1, NCLS]), ("dbg_M", [60, 520]),
                        ("dbg_wmT", [128, 4, 60]), ("dbg_R", [128, 4, NCLS]),
                        ("dbg_A2e", [128, 4, 21]), ("dbg_e2", [1, 21]),
                        ("dbg_sc", [128, 24]), ("dbg_u", [128, FEAT]),
                        ("dbg_utT", [128, FEAT]), ("dbg_soft", [128, 64])]:
            dbg[nm] = nc.dram_tensor(nm, shp, F32, kind="ExternalOutput").ap()

    qx_r = qx.rearrange("(t c p) f -> t p c f", c=NSUB, p=128)
    out_r = out_d.rearrange("(t c p) n -> t p c n", c=NSUB, p=128)

    with tile.TileContext(nc) as tc:
        with tc.tile_pool(name="persist", bufs=1) as pp, \
             tc.tile_pool(name="dram", bufs=1, space="DRAM") as dp:
            # ---- load constants/weights into SBUF -------------------------
            w1 = pp.tile([128, 4, HID], F32)
            nc.sync.dma_start(w1, w1_d.rearrange("(k p) n -> p k n", p=128))
            w2 = pp.tile([128, 2, FEAT], F32)
            nc.sync.dma_start(w2, w2_d.rearrange("(k p) n -> p k n", p=128))
            rw1 = pp.tile([128, 8, FEAT], F32)
            nc.sync.dma_start(rw1, rw1_d.rearrange("(k p) n -> p k n", p=128))
            rw2 = pp.tile([128, 4, FEAT], F32)
            nc.sync.dma_start(rw2, rw2_d.rearrange("(k p) n -> p k n", p=128))

            def colvec(src, k):
                t = pp.tile([128, k], F32, tag=f"cv_{src.tensor.name}")
                nc.sync.dma_start(t, src.rearrange("(k p) -> p k", p=128))
                return t

            b1T = colvec(b1_d, 2)
            b2T = colvec(b2_d, 4)
            rb1T = colvec(rb1_d, 4)
            rb2sT = colvec(rb2s_d, 4)
            g_sb = colvec(g_d, 4)
            b_sb = colvec(b_d, 4)
            p2g2s = colvec(p2g2_d, 4)
            p2gbs = colvec(p2gb_d, 4)
            m2g2s = colvec(m2g2_d, 4)
            m2gbs = colvec(m2gb_d, 4)

            ident = pp.tile([128, 128], F32)
            nc.sync.dma_start(ident, iden)
            ones8 = pp.tile([128, 8], mmdt)
            nc.sync.dma_start(ones8, ones8_d) if not bf else None
            if bf:
                o8f = pp.tile([128, 8], F32)
                nc.sync.dma_start(o8f, ones8_d)
                nc.scalar.copy(ones8, o8f)
            mask = pp.tile([128, 1], F32)
            nc.sync.dma_start(mask, mask_d)
            brows = pp.tile([1, 4, 128], F32)
            nc.sync.dma_start(brows, brows_d)
            oh_sb = pp.tile([128, 2, NCLS], F32)
            nc.sync.dma_start(oh_sb, oh.rearrange("(k p) c -> p k c", p=128))
            sup_sb = pp.tile([128, 2, FEAT], F32)
            nc.sync.dma_start(sup_sb, sup.rearrange("(k p) f -> p k f", p=128))

            onescol = pp.tile([128, 1], F32)
            nc.vector.memset(onescol, 1.0)
            ones1f_su = pp.tile([1, 128], F32)
            nc.vector.memset(ones1f_su, 1.0)
            epsc = pp.tile([128, 1], F32)
            nc.vector.memset(epsc, EPS)
            zeroc = pp.tile([128, 1], F32)
            nc.vector.memset(zeroc, 0.0)
            ones1r = pp.tile([1, 128], mmdt)
            nc.vector.memset(ones1r, 1.0)

            # persistent results of setup
            A_T = pp.tile([128, 4, NCLS], mmdt)      # 2*g*P, feature-major
            e_sb = pp.tile([1, NCLS], mmdt)          # -||P_c||^2 + 2 b.P_c
            P_T = pp.tile([128, 4, NCLS], F32)       # protos (g,b applied)
            wmT = pp.tile([128, 4, 60], F32)         # weighted means, T layout
            A2e = pp.tile([128, 4, 21], mmdt)        # [-2*g*R | 2gb]
            e2_sb = pp.tile([1, 21], mmdt)
            e_bc = pp.tile([128, NCLS], F32)         # e broadcast over partitions
            e2_bc = pp.tile([128, 21], F32)

            # ======================= SETUP: protos =========================
            with tc.tile_pool(name="su_ps", bufs=1, space="PSUM") as sps, \
                 tc.tile_pool(name="su_sb", bufs=2) as ssb:
                # LN support (2 row-chunks of 128)
                st6 = ssb.tile([128, 2, 6], F32)
                mv = ssb.tile([128, 2, 2], F32)
                for k in range(2):
                    nc.vector.bn_stats(st6[:, k, :], sup_sb[:, k, :])
                    nc.vector.bn_aggr(mv[:, k, :], st6[:, k, :])
                sd = ssb.tile([128, 2], F32)
                nc.scalar.activation(sd, mv[:, :, 1], AF.Sqrt, bias=epsc, scale=1.0)
                rr = ssb.tile([128, 2], F32)
                nc.vector.reciprocal(rr, sd)
                us = ssb.tile([128, 2, FEAT], F32)
                for k in range(2):
                    nmr = ssb.tile([128, 1], F32)
                    nc.vector.tensor_tensor(
                        out=nmr, in0=mv[:, k, 0:1], in1=rr[:, k:k + 1], op=ALU.mult)
                    nc.vector.tensor_scalar_mul(nmr, nmr, -1.0)
                    nc.gpsimd.tensor_scalar(
                        out=us[:, k, :], in0=sup_sb[:, k, :],
                        scalar1=rr[:, k:k + 1], scalar2=nmr,
                        op0=ALU.mult, op1=ALU.add)
                # cmeanT[f, c] = sum_s us[s, f] * oh[s, c]
                cmP = sps.tile([128, 4, NCLS], F32)
                for m in range(4):
                    for k in range(2):
                        nc.tensor.matmul(
                            cmP[:, m, :], lhsT=us[:, k, 128 * m:128 * (m + 1)],
                            rhs=oh_sb[:, k, :], start=(k == 0), stop=(k == 1))
                cmT = ssb.tile([128, 4, NCLS], F32)
                nc.scalar.copy(cmT, cmP)
                # h1T = relu(W1^T cmean + b1)
                h1P = sps.tile([128, 2, NCLS], F32)
                for m in range(2):
                    for k in range(4):
                        nc.tensor.matmul(
                            h1P[:, m, :], lhsT=w1[:, k, 128 * m:128 * (m + 1)],
                            rhs=cmT[:, k, :], start=(k == 0), stop=(k == 3))
                h1T = ssb.tile([128, 2, NCLS], F32)
                for m in range(2):
                    nc.scalar.activation(
                        h1T[:, m, :], h1P[:, m, :], AF.Relu,
                        bias=b1T[:, m:m + 1], scale=1.0)
                # p0T = W2^T h1 + b2
                p0P = sps.tile([128, 4, NCLS], F32)
                for m in range(4):
                    for k in range(2):
                        nc.tensor.matmul(
                            p0P[:, m, :], lhsT=w2[:, k, 128 * m:128 * (m + 1)],
                            rhs=h1T[:, k, :], start=(k == 0), stop=(k == 1))
                p0T = ssb.tile([128, 4, NCLS], F32)
                for m in range(4):
                    nc.scalar.activation(
                        p0T[:, m, :], p0P[:, m, :], AF.Identity,
                        bias=b2T[:, m:m + 1], scale=1.0)

                upT = ssb.tile([128, 4, NCLS], F32)
                _ln_over_partitions(nc, tc, sps, ssb, p0T, upT, onescol, ones1f_su,
                                    epsc, zeroc)
                for m in range(4):
                    nc.scalar.activation(
                        P_T[:, m, :], upT[:, m, :], AF.Identity,
                        bias=b_sb[:, m:m + 1], scale=g_sb[:, m:m + 1])
                    nc.scalar.activation(
                        A_T[:, m, :], upT[:, m, :], AF.Identity,
                        bias=p2gbs[:, m:m + 1], scale=p2g2s[:, m:m + 1])
                _class_consts(nc, sps, ssb, P_T, b_sb, onescol, e_sb,
                              scale_bp=2.0, sign_pp=-1.0, ident=ident,
                              zeroc=zeroc)
                ebP = sps.tile([128, 64], F32, tag="ebP")
                nc.tensor.matmul(ebP[:, 0:NCLS], lhsT=ones1f_su,
                                 rhs=e_sb if not bf else None, start=True,
                                 stop=True) if not bf else None
                if bf:
                    e_f32 = ssb.tile([1, NCLS], F32)
                    nc.scalar.copy(e_f32, e_sb)
                    nc.tensor.matmul(ebP[:, 0:NCLS], lhsT=ones1f_su, rhs=e_f32,
                                     start=True, stop=True)
                nc.scalar.copy(e_bc, ebP[:, 0:NCLS])
                if debug:
                    nc.sync.dma_start(dbg["dbg_P"], P_T)
                    nc.sync.dma_start(dbg["dbg_A"], A_T)
                    nc.sync.dma_start(dbg["dbg_e"], e_sb)

            # ======================= PASS 1 ================================
            with tc.tile_pool(name="p1x", bufs=3) as xp, \
                 tc.tile_pool(name="p1u", bufs=3) as up_, \
                 tc.tile_pool(name="p1ut", bufs=3) as utp, \
                 tc.tile_pool(name="p1sm", bufs=16) as smp, \
                 tc.tile_pool(name="p1soft", bufs=4) as sfp, \
                 tc.tile_pool(name="p1tp", bufs=2, space="PSUM") as tpp, \
                 tc.tile_pool(name="p1sc", bufs=4, space="PSUM") as scp, \
                 tc.tile_pool(name="p1acc", bufs=1, space="PSUM") as accp:
                pmacc = accp.tile([60, FEAT], F32)
                pws = accp.tile([60, 8], F32)
                nsub_tot = NST * NSUB
                for st_i in range(NST):
                    xt = xp.tile([128, NSUB, FEAT], F32)
                    nc.sync.dma_start(xt, qx_r[st_i])
                    r4, nmr4 = _ln_stats(nc, smp, xt, epsc)
                    ut = up_.tile([128, NSUB, FEAT], mmdt)
                    for c in range(NSUB):
                        nc.gpsimd.tensor_scalar(
                            out=ut[:, c, :], in0=xt[:, c, :],
                            scalar1=r4[:, c:c + 1], scalar2=nmr4[:, c:c + 1],
                            op0=ALU.mult, op1=ALU.add)
                    if mm_mode == "bf16":
                        utTall = _transpose_supertile(nc, utp, ut, mmdt)
                    for c in range(NSUB):
                        isub = st_i * NSUB + c
                        if mm_mode == "bf16":
                            utT = [utTall[:, 4 * c + j, :] for j in range(4)]
                        else:
                            utT = _transpose512(nc, tpp, utp, ut[:, c, :],
                                                ident, mmdt, mm_mode)
                        psc = scp.tile([128, 24], F32)
                        for j in range(4):
                            nc.tensor.matmul(
                                psc[:, 0:NCLS],
                                lhsT=_mm_cast(utT[j], mm_mode),
                                rhs=_mm_cast(A_T[:, j, :], mm_mode),
                                start=(j == 0), stop=(j == 3))
                        if debug and isub == 0:
                            scs = pp.tile([128, 24], F32)
                            nc.scalar.copy(scs, psc)
                            nc.sync.dma_start(dbg["dbg_sc"], scs)
                        sc2 = smp.tile([128, NCLS], F32, tag="sc2")
                        nc.vector.tensor_tensor(
                            out=sc2, in0=psc[:, 0:NCLS], in1=e_bc, op=ALU.add)
                        mx = smp.tile([128, 1], F32)
                        nc.vector.tensor_reduce(
                            mx, sc2, axis=mybir.AxisListType.X, op=ALU.max)
                        sc0 = smp.tile([128, NCLS], F32, tag="sc0")
                        nc.vector.tensor_scalar_sub(sc0, sc2, mx)
                        soft = sfp.tile([128, 64], mmdt)
                        for k in range(STEPS):
                            ek = smp.tile([128, NCLS], F32)
                            sk = smp.tile([128, 1], F32)
                            nc.scalar.activation(
                                ek, sc0, AF.Exp,
                                bias=zeroc, scale=1.0 / float(k + 1),
                                accum_out=sk)
                            rk = smp.tile([128, 1], F32)
                            nc.vector.reciprocal(rk, sk)
                            nc.gpsimd.tensor_scalar_mul(
                                soft[:, NCLS * k:NCLS * (k + 1)], ek, rk)
                        if debug and isub == 0:
                            nc.sync.dma_start(dbg["dbg_soft"], soft)
                        if st_i == NST - 1 and c == NSUB - 1:
                            nc.gpsimd.tensor_scalar_mul(
                                soft[:, 0:60], soft[:, 0:60], mask)
                        nc.tensor.matmul(
                            pmacc, lhsT=_mm_cast(soft[:, 0:60], mm_mode),
                            rhs=_mm_cast(ut[:, c, :], mm_mode),
                            start=(isub == 0), stop=(isub == nsub_tot - 1))
                        nc.tensor.matmul(
                            pws[:, 0:8], lhsT=_mm_cast(soft[:, 0:60], mm_mode),
                            rhs=ones8, start=(isub == 0),
                            stop=(isub == nsub_tot - 1))
                # ship partial sums to the collective (PSUM -> SBUF -> DRAM)
                bin_ = dp.tile([60, 520], F32)
                bout = dp.tile([60, 520], F32)
                stage = pp.tile([60, 520], F32)
                nc.scalar.copy(stage[:, 0:FEAT], pmacc)
                nc.vector.tensor_copy(out=stage[:, FEAT:520], in_=pws)
                nc.sync.dma_start(bin_, stage)

            nc.gpsimd.collective_compute(
                "AllReduce", ALU.add,
                replica_groups=[list(range(NCORES))],
                ins=[bin_.opt()], outs=[bout.opt()])

            # ================== MID: wmeans + refine chain =================
            with tc.tile_pool(name="md_ps", bufs=1, space="PSUM") as mps, \
                 tc.tile_pool(name="md_sb", bufs=2) as msb:
                Mw = msb.tile([60, 520], F32)
                nc.sync.dma_start(Mw, bout)
                ws = msb.tile([60, 1], F32)
                nc.vector.tensor_scalar_max(ws, Mw[:, FEAT:FEAT + 1], 1e-6)
                rw60 = msb.tile([60, 1], F32)
                nc.vector.reciprocal(rw60, ws)
                sr = msb.tile([60, 1], F32)
                nc.vector.tensor_tensor(
                    out=sr, in0=Mw[:, FEAT:FEAT + 1], in1=rw60, op=ALU.mult)
                # transpose M and the two per-class vectors
                mtP = mps.tile([128, 4, 60], F32)
                for j in range(4):
                    nc.tensor.transpose(
                        mtP[:, j, :], Mw[0:60, 128 * j:128 * (j + 1)],
                        ident[0:60, 0:60])
                rsP = mps.tile([1, 128], F32)
                nc.tensor.transpose(rsP[0:1, 0:60], rw60, ident[0:60, 0:60])
                nc.tensor.transpose(rsP[0:1, 64:124], sr, ident[0:60, 0:60])
                rsT = msb.tile([1, 128], F32)
                nc.scalar.copy(rsT[0:1, 0:60], rsP[0:1, 0:60])
                nc.scalar.copy(rsT[0:1, 64:124], rsP[0:1, 64:124])
                # broadcast across partitions via K=1 matmuls
                ones1f = msb.tile([1, 128], F32)
                nc.vector.memset(ones1f, 1.0)
                bcP = mps.tile([128, 128], F32)
                nc.tensor.matmul(bcP[:, 0:60], lhsT=ones1f, rhs=rsT[0:1, 0:60],
                                 start=True, stop=True)
                bsrP = mps.tile([128, 4, 60], F32)
                for m in range(4):
                    nc.tensor.matmul(
                        bsrP[:, m, :], lhsT=brows[0:1, m, :],
                        rhs=rsT[0:1, 64:124], start=True, stop=True)
                rwbc = msb.tile([128, 60], F32)
                nc.scalar.copy(rwbc, bcP[:, 0:60])
                bsr = msb.tile([128, 4, 60], F32)
                nc.scalar.copy(bsr, bsrP)
                for m in range(4):
                    t1 = msb.tile([128, 60], F32)
                    nc.vector.tensor_tensor(
                        out=t1, in0=mtP[:, m, :], in1=rwbc, op=ALU.mult)
                    t2 = msb.tile([128, 60], F32)
                    nc.scalar.activation(t2, t1, AF.Identity,
                                         bias=zeroc, scale=g_sb[:, m:m + 1])
                    nc.vector.tensor_tensor(
                        out=wmT[:, m, :], in0=t2, in1=bsr[:, m, :], op=ALU.add)

                if debug:
                    nc.sync.dma_start(dbg["dbg_M"], Mw)
                refT = msb.tile([128, 4, NCLS], F32)
                nc.scalar.copy(refT, P_T)
                for step in range(STEPS):
                    hP = mps.tile([128, 4, NCLS], F32)
                    for m in range(4):
                        for kk in range(8):
                            rhs = (refT[:, kk, :] if kk < 4 else
                                   wmT[:, kk - 4, NCLS * step:NCLS * (step + 1)])
                            nc.tensor.matmul(
                                hP[:, m, :],
                                lhsT=rw1[:, kk, 128 * m:128 * (m + 1)],
                                rhs=rhs, start=(kk == 0), stop=(kk == 7))
                    hT = msb.tile([128, 4, NCLS], F32)
                    for m in range(4):
                        nc.scalar.activation(hT[:, m, :], hP[:, m, :], AF.Relu,
                                             bias=rb1T[:, m:m + 1], scale=1.0)
                    dP = mps.tile([128, 4, NCLS], F32)
                    for m in range(4):
                        for kk in range(4):
                            nc.tensor.matmul(
                                dP[:, m, :],
                                lhsT=rw2[:, kk, 128 * m:128 * (m + 1)],
                                rhs=hT[:, kk, :], start=(kk == 0), stop=(kk == 3))
                    refT_new = msb.tile([128, 4, NCLS], F32)
                    for m in range(4):
                        t = msb.tile([128, NCLS], F32)
                        nc.scalar.activation(t, dP[:, m, :], AF.Identity,
                                             bias=rb2sT[:, m:m + 1], scale=0.1)
                        nc.vector.tensor_tensor(
                            out=refT_new[:, m, :], in0=refT[:, m, :], in1=t,
                            op=ALU.add)
                    refT = refT_new

                upRT = msb.tile([128, 4, NCLS], F32)
                _ln_over_partitions(nc, tc, mps, msb, refT, upRT, onescol, ones1f,
                                    epsc, zeroc)
                R_T = msb.tile([128, 4, NCLS], F32)
                for m in range(4):
                    nc.scalar.activation(
                        R_T[:, m, :], upRT[:, m, :], AF.Identity,
                        bias=b_sb[:, m:m + 1], scale=g_sb[:, m:m + 1])
                    nc.scalar.activation(
                        A2e[:, m, 0:NCLS], upRT[:, m, :], AF.Identity,
                        bias=m2gbs[:, m:m + 1], scale=m2g2s[:, m:m + 1])
                    nc.vector.tensor_copy(
                        out=A2e[:, m, NCLS:21], in_=p2gbs[:, m:m + 1])
                _class_consts(nc, mps, msb, R_T, b_sb, onescol, e2_sb[0:1, 0:NCLS],
                              scale_bp=-2.0, sign_pp=1.0, ident=ident,
                              zeroc=zeroc)
                nc.vector.memset(e2_sb[0:1, NCLS:21], bnorm2)
                e2bP = mps.tile([128, 64], F32, tag="bcP")
                if bf:
                    e2_f32 = msb.tile([1, 21], F32)
                    nc.scalar.copy(e2_f32, e2_sb)
                    nc.tensor.matmul(e2bP[:, 0:21], lhsT=ones1f,
                                     rhs=e2_f32, start=True, stop=True)
                else:
                    nc.tensor.matmul(e2bP[:, 0:21], lhsT=ones1f,
                                     rhs=e2_sb, start=True, stop=True)
                nc.scalar.copy(e2_bc, e2bP[:, 0:21])
                if debug:
                    nc.sync.dma_start(dbg["dbg_wmT"], wmT)
                    nc.sync.dma_start(dbg["dbg_R"], R_T)
                    nc.sync.dma_start(dbg["dbg_A2e"], A2e)
                    nc.sync.dma_start(dbg["dbg_e2"], e2_sb)

            # ======================= PASS 2 ================================
            with tc.tile_pool(name="p2x", bufs=3) as xp2, \
                 tc.tile_pool(name="p2u", bufs=3) as up2, \
                 tc.tile_pool(name="p2ut", bufs=3) as utp2, \
                 tc.tile_pool(name="p2sm", bufs=16) as smp2, \
                 tc.tile_pool(name="p2l", bufs=3) as lp2, \
                 tc.tile_pool(name="p2tp", bufs=2, space="PSUM") as tpp2, \
                 tc.tile_pool(name="p2z", bufs=4, space="PSUM") as zp2:
                for st_i in range(NST):
                    xt = xp2.tile([128, NSUB, FEAT], F32)
                    nc.sync.dma_start(xt, qx_r[st_i])
                    r4, nmr4, v4 = _ln_stats(nc, smp2, xt, epsc, want_v=True)
                    nrm4 = smp2.tile([128, NSUB], F32)
                    nc.vector.tensor_tensor(out=nrm4, in0=v4, in1=r4, op=ALU.mult)
                    nc.vector.tensor_tensor(out=nrm4, in0=nrm4, in1=r4, op=ALU.mult)
                    nc.vector.tensor_scalar_mul(nrm4, nrm4, float(FEAT) * gamma2)
                    ut = up2.tile([128, NSUB, FEAT], mmdt)
                    for c in range(NSUB):
                        nc.gpsimd.tensor_scalar(
                            out=ut[:, c, :], in0=xt[:, c, :],
                            scalar1=r4[:, c:c + 1], scalar2=nmr4[:, c:c + 1],
                            op0=ALU.mult, op1=ALU.add)
                    lt = lp2.tile([128, NSUB, NCLS], F32)
                    if mm_mode == "bf16":
                        utTall = _transpose_supertile(nc, utp2, ut, mmdt)
                    for c in range(NSUB):
                        if mm_mode == "bf16":
                            utT = [utTall[:, 4 * c + j, :] for j in range(4)]
                        else:
                            utT = _transpose512(nc, tpp2, utp2, ut[:, c, :],
                                                ident, mmdt, mm_mode)
                        pz = zp2.tile([128, 24], F32)
                        for j in range(4):
                            nc.tensor.matmul(
                                pz[:, 0:21],
                                lhsT=_mm_cast(utT[j], mm_mode),
                                rhs=_mm_cast(A2e[:, j, :], mm_mode),
                                start=(j == 0), stop=(j == 3))
                        z2 = smp2.tile([128, 21], F32, tag="z2")
                        nc.vector.tensor_tensor(
                            out=z2, in0=pz[:, 0:21], in1=e2_bc, op=ALU.add)
                        bias2 = smp2.tile([128, 1], F32)
                        nc.vector.tensor_tensor(
                            out=bias2, in0=nrm4[:, c:c + 1], in1=z2[:, NCLS:21],
                            op=ALU.add)
                        nc.scalar.activation(lt[:, c, :], z2[:, 0:NCLS], AF.Relu,
                                             bias=bias2, scale=1.0)
                    nc.gpsimd.tensor_scalar_mul(lt, lt, -float(dist_temp))
                    nc.sync.dma_start(out_r[st_i], lt)

    nc.compile()
    return nc


def _ln_stats(nc, pool, xt, epsc, want_v=False):
    """bn_stats LN statistics for a [128, NSUB, FEAT] tile.

    Returns r (rstd), nmr (-mean*rstd) as [128, NSUB], optionally v."""
    st6 = pool.tile([128, NSUB, 6], F32)
    mv = pool.tile([128, NSUB, 2], F32)
    for c in range(NSUB):
        nc.vector.bn_stats(st6[:, c, :], xt[:, c, :])
        nc.vector.bn_aggr(mv[:, c, :], st6[:, c, :])
    sd = pool.tile([128, NSUB], F32)
    nc.scalar.activation(sd, mv[:, :, 1], AF.Sqrt, bias=epsc, scale=1.0)
    r4 = pool.tile([128, NSUB], F32)
    nc.vector.reciprocal(r4, sd)
    nmr4 = pool.tile([128, NSUB], F32)
    nc.vector.tensor_tensor(out=nmr4, in0=mv[:, :, 0], in1=r4, op=ALU.mult)
    nc.vector.tensor_scalar_mul(nmr4, nmr4, -1.0)
    if want_v:
        v4 = pool.tile([128, NSUB], F32)
        nc.vector.tensor_copy(out=v4, in_=mv[:, :, 1])
        return r4, nmr4, v4
    return r4, nmr4


def _transpose_supertile(nc, sbuf_pool, ut, mmdt):
    """bf16: one xbar transpose for all 4 subtiles: [128, 4, 512] ->
    [128, 16, 128]; subtile c's feature-chunk j lives at [:, 4*c+j, :]."""
    utTall = sbuf_pool.tile([128, 16, 128], mmdt, tag="utTall")
    nc.scalar.dma_start_transpose(utTall, ut)
    return utTall


def _transpose512(nc, psum_pool, sbuf_pool, src, ident, mmdt, mm_mode):
    """[128q, 512f] SBUF -> 4 chunk APs [128f_j, 128q] (feature-major)."""
    pT = psum_pool.tile([128, FEAT], mmdt)
    for j in range(4):
        inp = src[:, 128 * j:128 * (j + 1)]
        nc.tensor.transpose(pT[:, 128 * j:128 * (j + 1)], inp, ident)
    utT = sbuf_pool.tile([128, FEAT], mmdt, tag="utT")
    nc.scalar.copy(utT[:, 0:256], pT[:, 0:256])
    nc.vector.tensor_copy(out=utT[:, 256:FEAT], in_=pT[:, 256:FEAT])
    return [utT[:, 128 * j:128 * (j + 1)] for j in range(4)]


def _ln_over_partitions(nc, tc, psum_pool, sbuf_pool, xT, outT, onescol, ones1f,
                        epsc, zeroc):
    """LayerNorm over the partition axis for [128, 4, NCLS] (512 features of
    NCLS classes, feature-major). outT = (x - mean) * rstd."""
    sP = psum_pool.tile([1, 64], F32)
    x2 = sbuf_pool.tile([128, 4, NCLS], F32)
    nc.scalar.activation(x2, xT, AF.Square, bias=zeroc, scale=1.0)
    for m in range(4):
        nc.tensor.matmul(sP[0:1, 0:NCLS], lhsT=onescol, rhs=xT[:, m, :],
                         start=(m == 0), stop=(m == 3))
    for m in range(4):
        nc.tensor.matmul(sP[0:1, 32:32 + NCLS], lhsT=onescol, rhs=x2[:, m, :],
                         start=(m == 0), stop=(m == 3))
    mrow = sbuf_pool.tile([1, NCLS], F32)
    nc.vector.tensor_scalar_mul(mrow, sP[0:1, 0:NCLS], 1.0 / FEAT)
    m2 = sbuf_pool.tile([1, NCLS], F32)
    nc.scalar.activation(m2, mrow, AF.Square, bias=zeroc[0:1], scale=1.0)
    vrow = sbuf_pool.tile([1, NCLS], F32)
    nc.vector.tensor_scalar(out=vrow, in0=sP[0:1, 32:32 + NCLS],
                            scalar1=1.0 / FEAT, scalar2=None, op0=ALU.mult)
    nc.vector.tensor_tensor(out=vrow, in0=vrow, in1=m2, op=ALU.subtract)
    sdr = sbuf_pool.tile([1, NCLS], F32)
    nc.scalar.activation(sdr, vrow, AF.Sqrt, bias=epsc[0:1], scale=1.0)
    rrow = sbuf_pool.tile([1, NCLS], F32)
    nc.vector.reciprocal(rrow, sdr)
    bcP = psum_pool.tile([128, 64], F32)
    nc.tensor.matmul(bcP[:, 0:NCLS], lhsT=ones1f, rhs=mrow, start=True, stop=True)
    nc.tensor.matmul(bcP[:, 32:32 + NCLS], lhsT=ones1f, rhs=rrow,
                     start=True, stop=True)
    mbc = sbuf_pool.tile([128, NCLS], F32)
    nc.scalar.copy(mbc, bcP[:, 0:NCLS])
    rbc = sbuf_pool.tile([128, NCLS], F32)
    nc.scalar.copy(rbc, bcP[:, 32:32 + NCLS])
    for m in range(4):
        nc.vector.tensor_tensor(out=outT[:, m, :], in0=xT[:, m, :], in1=mbc,
                                op=ALU.subtract)
        nc.vector.tensor_tensor(out=outT[:, m, :], in0=outT[:, m, :], in1=rbc,
                                op=ALU.mult)


def _class_consts(nc, psum_pool, sbuf_pool, P_T, b_sb, onescol, e_out,
                  scale_bp, sign_pp, ident, zeroc):
    """e_out[1, NCLS] = sign_pp * ||P_c||^2 + scale_bp * (b . P_c)."""
    PT2 = sbuf_pool.tile([128, 4, NCLS], F32)
    nc.scalar.activation(PT2, P_T, AF.Square, bias=zeroc, scale=1.0)
    eP = psum_pool.tile([1, 64], F32)
    for m in range(4):
        nc.tensor.matmul(eP[0:1, 0:NCLS], lhsT=onescol, rhs=PT2[:, m, :],
                         start=(m == 0), stop=(m == 3))
    for m in range(4):
        nc.tensor.matmul(eP[0:1, 32:32 + NCLS], lhsT=b_sb[:, m:m + 1],
                         rhs=P_T[:, m, :], start=(m == 0), stop=(m == 3))
    t = sbuf_pool.tile([1, NCLS], F32)
    nc.vector.tensor_scalar_mul(t, eP[0:1, 32:32 + NCLS], scale_bp)
    t2 = sbuf_pool.tile([1, NCLS], F32)
    nc.vector.tensor_scalar_mul(t2, eP[0:1, 0:NCLS], sign_pp)
    nc.vector.tensor_tensor(out=e_out, in0=t2, in1=t, op=ALU.add)


def kernel(_debug=False, **inputs) -> np.ndarray:
    global LAST_EXEC_NS, LAST_RESULTS
    f32 = np.float32
    qf = np.asarray(inputs["query_features"], f32)
    sf = np.asarray(inputs["support_features"], f32)
    lab = np.asarray(inputs["support_labels"]).astype(np.int64)
    g = np.asarray(inputs["ln_g"], f32)
    b = np.asarray(inputs["ln_b"], f32)
    temp = float(np.asarray(inputs["dist_temp"]))

    assert np.allclose(g, g[0]), "kernel fast path assumes constant ln_g"
    gamma2 = float(g[0]) ** 2
    bnorm2 = float(b @ b)

    nc = build_graph(gamma2, bnorm2, temp, MM_MODE, debug=_debug)

    sup = np.zeros((256, FEAT), f32)
    sup[:NSUP] = sf
    cnt = np.bincount(lab, minlength=NCLS).astype(f32)
    oh = np.zeros((256, NCLS), f32)
    oh[np.arange(NSUP), lab] = 1.0 / np.maximum(cnt, 1.0)[lab]
    mask = (np.arange(128) < LAST_VALID).astype(f32).reshape(128, 1)

    common = {
        "sup": sup, "oh": oh,
        "iden": np.eye(128, dtype=f32),
        "ones8": np.ones((128, 8), f32),
        "mask": mask,
        "g": g, "b": b,
        "p2g2": (2.0 * g * g).astype(f32), "p2gb": (2.0 * g * b).astype(f32),
        "m2g2": (-2.0 * g * g).astype(f32), "m2gb": (-2.0 * g * b).astype(f32),
        "brows": b.reshape(1, 4, 128).copy(),
        "w1": np.asarray(inputs["pg_w1"], f32), "b1": np.asarray(inputs["pg_b1"], f32),
        "w2": np.asarray(inputs["pg_w2"], f32), "b2": np.asarray(inputs["pg_b2"], f32),
        "rw1": np.asarray(inputs["rf_w1"], f32), "rb1": np.asarray(inputs["rf_b1"], f32),
        "rw2": np.asarray(inputs["rf_w2"], f32),
        "rb2s": (0.1 * np.asarray(inputs["rf_b2"], f32)).astype(f32),
    }
    in_maps = []
    for i in range(NCORES):
        shard = np.zeros((NQP, FEAT), f32)
        shard[:NQL] = qf[i * NQL:(i + 1) * NQL]
        in_maps.append({"qx": shard, **common})

    trace = bool(int(os.environ.get("KERNEL_TRACE", "0")))
    res = run_bass_kernel_spmd(nc, in_maps, list(range(NCORES)), trace=trace)
    LAST_EXEC_NS = res.exec_time_ns
    LAST_RESULTS = res
    out = np.concatenate([res.results[i]["out"][:NQL] for i in range(NCORES)], 0)
    return out
